# revision 3
# baseline (speedup 1.0000x reference)
import sys

sys.path.insert(0, "/opt/trn_rl_repo")

import numpy as np
import ml_dtypes

# ---- problem constants (hardcoded from the nn_LocalAggregator spec) ----
PC_MIN = np.array([-40.0, -40.0, -1.0], dtype=np.float32)
GRID = np.float32(0.4)
SCALE_MULT = np.float32(3.0)
N_PTS, N_GAUSS, N_CLS = 16384, 4096, 18
N_CORES = 8
NPC = N_PTS // N_CORES          # 2048 points per core
BLK = 512                       # point block (matmul free dim)
NBLK = NPC // BLK               # 4
P = 128                         # partitions / gaussians per tile
BIG = np.float64(1024.0)        # mask penalty (one violated axis is enough)
DUMMY_BIAS = -30000.0           # exp(-30000) == 0 exactly in fp32
KQ = 9                          # quadratic+linear monomial features
NSPLIT = [(0, 0), (0, 1), (1, 0), (1, 1), (0, 2), (2, 0)]  # bf16 split combos
KQR = KQ * len(NSPLIT)          # quad rows after splitting
KZ = 16                         # z voxel range
Y_CULL = True

BF16 = ml_dtypes.bfloat16

# module global for test harness introspection (exec time etc.)
LAST_RESULTS = None


def _split3(x):
    """float64 array -> 3 bf16 levels whose sum ~= x to ~24 bits."""
    a = x.astype(BF16)
    r = x - a.astype(np.float64)
    b = r.astype(BF16)
    r = r - b.astype(np.float64)
    c = r.astype(BF16)
    return a, b, c


def _prep(pts, means3D, opacities, semantics, scales, cov3D):
    """Host-side O(N+M) prep: sharding, features, coefficient tables."""
    p = np.asarray(pts[0], dtype=np.float32)          # [N,3]
    mu = np.asarray(means3D[0], dtype=np.float32)     # [M,3]
    opa = np.asarray(opacities[0], dtype=np.float32)  # [M]
    sem = np.asarray(semantics[0], dtype=np.float32)  # [M,C]
    sc = np.asarray(scales[0], dtype=np.float32)      # [M,3]
    cov = np.asarray(cov3D[0], dtype=np.float32)      # [M,3,3]

    # integer voxel coords / radii -- fp32 ops exactly as the reference
    p_int = ((p - PC_MIN) / GRID).astype(np.int32)
    m_int = ((mu - PC_MIN) / GRID).astype(np.int32)
    radii = np.ceil(sc.max(axis=-1) * SCALE_MULT / GRID).astype(np.int32)

    # symmetric precision entries, same picks as reference
    cxx = cov[:, 0, 0].astype(np.float64)
    cyy = cov[:, 1, 1].astype(np.float64)
    czz = cov[:, 2, 2].astype(np.float64)
    cxy = cov[:, 0, 1].astype(np.float64)
    cyz = cov[:, 1, 2].astype(np.float64)
    cxz = cov[:, 0, 2].astype(np.float64)
    with np.errstate(divide="ignore"):
        lnopa = np.log(opa.astype(np.float64))
    lnopa = np.maximum(lnopa, -20000.0)

    # ---- shard points: equal x-chunks, y-sorted inside each core ----
    order_x = np.argsort(p[:, 0], kind="stable")
    core_idx = []
    for c in range(N_CORES):
        idx = order_x[c * NPC:(c + 1) * NPC]
        idx = idx[np.argsort(p[idx, 1], kind="stable")]
        core_idx.append(idx)

    # ---- per-core gaussian subsets (x-reach cull), y-sorted ----
    core_gsel = []
    for c in range(N_CORES):
        vx = p_int[core_idx[c], 0]
        m = (m_int[:, 0] >= vx.min() - radii) & (m_int[:, 0] <= vx.max() + radii)
        gs = np.nonzero(m)[0]
        gs = gs[np.argsort(m_int[gs, 1], kind="stable")]
        core_gsel.append(gs)
    g_pad = P * int(np.ceil(max(len(g) for g in core_gsel) / P))
    n_gt = g_pad // P

    # ---- structural sizes shared across cores (SPMD) ----
    kx = 0
    ky = 0
    for c in range(N_CORES):
        vx = p_int[core_idx[c], 0]
        kx = max(kx, int(vx.max() - vx.min()) + 1)
        vy = p_int[core_idx[c], 1]
        for b in range(NBLK):
            vyb = vy[b * BLK:(b + 1) * BLK]
            ky = max(ky, int(vyb.max() - vyb.min()) + 1)
    ktot = KQR + kx + ky + KZ

    # ---- pair lists (g-tile x block), y-culled, padded across cores ----
    core_pairs = []   # per core: list of (b, t) with t = -1 for dummy
    counts = np.zeros((N_CORES, NBLK), dtype=np.int64)
    per_core_tb = []
    for c in range(N_CORES):
        gs = core_gsel[c]
        vy = p_int[core_idx[c], 1]
        tb = []
        for b in range(NBLK):
            vyb = vy[b * BLK:(b + 1) * BLK]
            ylo, yhi = int(vyb.min()), int(vyb.max())
            lst = []
            for t in range(n_gt):
                gg = gs[t * P:(t + 1) * P]
                if len(gg) == 0:
                    continue
                gl = (m_int[gg, 1] - radii[gg]).min()
                gh = (m_int[gg, 1] + radii[gg]).max()
                if (not Y_CULL) or (gl <= yhi and gh >= ylo):
                    lst.append(t)
            tb.append(lst)
            counts[c, b] = len(lst)
        per_core_tb.append(tb)
    npb = counts.max(axis=0)          # padded per-block pair counts
    npair = int(npb.sum())
    pair_block = []                    # baked structure, same for all cores
    for b in range(NBLK):
        pair_block += [b] * int(npb[b])

    # ---- per-core device arrays ----
    in_maps = []
    for c in range(N_CORES):
        idx = core_idx[c]
        gs = core_gsel[c]
        vx = p_int[idx, 0]
        vy = p_int[idx, 1]
        vz = p_int[idx, 2]
        vx_lo = int(vx.min())
        pc = p[idx].astype(np.float64)     # [NPC,3]

        # per-block centers
        centers = np.stack([pc[b * BLK:(b + 1) * BLK].mean(axis=0)
                            for b in range(NBLK)])   # [NBLK,3]
        ylos = [int(vy[b * BLK:(b + 1) * BLK].min()) for b in range(NBLK)]

        # ---- feature matrix FEAT [ktot, NPC] bf16 ----
        feat = np.zeros((ktot, NPC), dtype=BF16)
        for b in range(NBLK):
            cols = slice(b * BLK, (b + 1) * BLK)
            d = pc[cols] - centers[b]                  # [BLK,3] float64
            x, y, z = d[:, 0], d[:, 1], d[:, 2]
            q = np.stack([x * x, y * y, z * z, x * y, y * z, x * z, x, y, z])
            qs = _split3(q)                            # 3 x [KQ, BLK]
            for f in range(KQ):
                for k, (i, _) in enumerate(NSPLIT):
                    feat[f * len(NSPLIT) + k, cols] = qs[i][f]
            # one-hots
            rx = KQR + (vx[cols] - vx_lo)
            ryy = KQR + kx + (vy[cols] - ylos[b])
            rz = KQR + kx + ky + vz[cols]
            ar = np.arange(b * BLK, (b + 1) * BLK)
            feat[rx, ar] = BF16(1)
            feat[ryy, ar] = BF16(1)
            feat[rz, ar] = BF16(1)

        # ---- per-pair stationaries / biases / sem tiles ----
        stat = np.zeros((npair, ktot, P), dtype=BF16)
        bias = np.full((P, npair), DUMMY_BIAS, dtype=np.float32)
        semt = np.zeros((P, npair * N_CLS), dtype=np.float32)
        pi = 0
        for b in range(NBLK):
            lst = per_core_tb[c][b]
            for j in range(int(npb[b])):
                if j < len(lst):
                    t = lst[j]
                    gg = gs[t * P:(t + 1) * P]
                    ng = len(gg)
                    mup = mu[gg].astype(np.float64) - centers[b]  # [ng,3]
                    mx, my, mz = mup[:, 0], mup[:, 1], mup[:, 2]
                    gxx, gyy, gzz = cxx[gg], cyy[gg], czz[gg]
                    gxy, gyz, gxz = cxy[gg], cyz[gg], cxz[gg]
                    hx = gxx * mx + gxy * my + gxz * mz
                    hy = gxy * mx + gyy * my + gyz * mz
                    hz = gxz * mx + gyz * my + gzz * mz
                    gq = np.stack([-0.5 * gxx, -0.5 * gyy, -0.5 * gzz,
                                   -gxy, -gyz, -gxz, hx, hy, hz])  # [KQ,ng]
                    gsp = _split3(gq)
                    for f in range(KQ):
                        for k, (_, jj) in enumerate(NSPLIT):
                            stat[pi, f * len(NSPLIT) + k, :ng] = gsp[jj][f]
                    # interval tables (0 within reach, -BIG outside)
                    vv = np.arange(kx)[:, None] + vx_lo
                    out_x = np.abs(vv - m_int[gg, 0][None, :]) > radii[gg][None, :]
                    stat[pi, KQR:KQR + kx, :ng] = np.where(out_x, -BIG, 0.0).astype(BF16)
                    vv = np.arange(ky)[:, None] + ylos[b]
                    out_y = np.abs(vv - m_int[gg, 1][None, :]) > radii[gg][None, :]
                    stat[pi, KQR + kx:KQR + kx + ky, :ng] = np.where(out_y, -BIG, 0.0).astype(BF16)
                    vv = np.arange(KZ)[:, None]
                    out_z = np.abs(vv - m_int[gg, 2][None, :]) > radii[gg][None, :]
                    stat[pi, KQR + kx + ky:, :ng] = np.where(out_z, -BIG, 0.0).astype(BF16)
                    # bias: -0.5 mu'^T C mu' + ln(opa)
                    quad = (gxx * mx * mx + gyy * my * my + gzz * mz * mz
                            + 2 * gxy * mx * my + 2 * gyz * my * mz + 2 * gxz * mx * mz)
                    bias[:ng, pi] = (-0.5 * quad + lnopa[gg]).astype(np.float32)
                    semt[:ng, pi * N_CLS:(pi + 1) * N_CLS] = sem[gg]
                pi += 1

        # chunk layout: stat rows as [kchunks, 128, P] padded
        nchunks = int(np.ceil(ktot / P))
        kpad = nchunks * P
        featp = np.zeros((kpad, NPC), dtype=BF16)
        featp[:ktot] = feat
        statp = np.zeros((npair, kpad, P), dtype=BF16)
        statp[:, :ktot] = stat
        statt = statp.reshape(npair, nchunks, P, P)  # [pair, chunk, krow, g]
        statt = statt.transpose(2, 1, 0, 3).reshape(P, nchunks * npair * P)
        # rows = krow partition (128), cols = (chunk, pair, gauss)

        in_maps.append({
            "feat": featp.reshape(nchunks, P, NPC).transpose(1, 0, 2).reshape(P, nchunks * NPC),
            "stat": statt,
            "bias": bias,
            "semt": semt,
        })

    meta = dict(npair=npair, pair_block=pair_block, nchunks=nchunks,
                core_idx=core_idx, npb=npb)
    return in_maps, meta


def _build_nc(npair, pair_block, nchunks):
    import concourse.bass as bass  # noqa: F401
    import concourse.mybir as mybir
    import concourse.tile as tile
    from concourse import bacc

    f32 = mybir.dt.float32
    bf16 = mybir.dt.bfloat16

    nc = bacc.Bacc("TRN2", target_bir_lowering=False, debug=False,
                   num_devices=N_CORES)
    feat_d = nc.dram_tensor("feat", [P, nchunks * NPC], bf16, kind="ExternalInput")
    stat_d = nc.dram_tensor("stat", [P, nchunks * npair * P], bf16, kind="ExternalInput")
    bias_d = nc.dram_tensor("bias", [P, npair], f32, kind="ExternalInput")
    semt_d = nc.dram_tensor("semt", [P, npair * N_CLS], f32, kind="ExternalInput")
    out_d = nc.dram_tensor("out", [N_CLS, NPC], f32, kind="ExternalOutput")

    # first/last pair index per block for psum accumulate flags
    first = {}
    last = {}
    for i, b in enumerate(pair_block):
        first.setdefault(b, i)
        last[b] = i

    with tile.TileContext(nc) as tc:
        with (
            tc.tile_pool(name="resident", bufs=1) as res_pool,
            tc.tile_pool(name="wpool", bufs=3) as w_pool,
            tc.tile_pool(name="pw", bufs=2, space="PSUM") as pw_pool,
            tc.tile_pool(name="lgp", bufs=1, space="PSUM") as lg_pool,
        ):
            feat_s = res_pool.tile([P, nchunks * NPC], bf16, name="feat_s")
            stat_s = res_pool.tile([P, nchunks * npair * P], bf16, name="stat_s")
            bias_s = res_pool.tile([P, npair], f32, name="bias_s")
            semt_s = res_pool.tile([P, npair * N_CLS], f32, name="semt_s")
            out_s = res_pool.tile([N_CLS, NPC], f32, name="out_s")

            # stage inputs (split feat/stat DMAs for pipelining)
            nc.sync.dma_start(out=bias_s[:], in_=bias_d[:])
            nc.sync.dma_start(out=semt_s[:], in_=semt_d[:])
            for ch in range(nchunks):
                nc.sync.dma_start(
                    out=feat_s[:, ch * NPC:(ch + 1) * NPC],
                    in_=feat_d[:, ch * NPC:(ch + 1) * NPC])
            nstat = nchunks * npair
            for s in range(nstat):
                nc.sync.dma_start(
                    out=stat_s[:, s * P:(s + 1) * P],
                    in_=stat_d[:, s * P:(s + 1) * P])

            lg = [lg_pool.tile([N_CLS, BLK], f32, name=f"lg{b}")
                  for b in range(NBLK)]

            for i, b in enumerate(pair_block):
                cols = slice(b * BLK, (b + 1) * BLK)
                pw = pw_pool.tile([P, BLK], f32, name="pw")
                for ch in range(nchunks):
                    lhs = stat_s[:, (ch * npair + i) * P:(ch * npair + i + 1) * P]
                    rhs = feat_s[:, ch * NPC + b * BLK: ch * NPC + (b + 1) * BLK]
                    nc.tensor.matmul(out=pw[:], lhsT=lhs, rhs=rhs,
                                     start=(ch == 0), stop=(ch == nchunks - 1))
                w = w_pool.tile([P, BLK], f32, name="w")
                nc.scalar.activation(w[:], pw[:],
                                     mybir.ActivationFunctionType.Exp,
                                     bias=bias_s[:, i:i + 1])
                nc.tensor.matmul(out=lg[b][:],
                                 lhsT=semt_s[:, i * N_CLS:(i + 1) * N_CLS],
                                 rhs=w[:],
                                 start=(first[b] == i), stop=(last[b] == i))

            for b in range(NBLK):
                nc.vector.tensor_copy(out_s[:, b * BLK:(b + 1) * BLK], lg[b][:])
            nc.sync.dma_start(out=out_d[:], in_=out_s[:])

    nc.compile()
    return nc


def kernel(pts, means3D, opacities, semantics, scales, cov3D):
    global LAST_RESULTS
    from concourse.bass_utils import run_bass_kernel_spmd

    in_maps, meta = _prep(pts, means3D, opacities, semantics, scales, cov3D)
    nc = _build_nc(meta["npair"], meta["pair_block"], meta["nchunks"])
    res = run_bass_kernel_spmd(nc, in_maps, core_ids=list(range(N_CORES)))
    LAST_RESULTS = res

    out = np.empty((N_PTS, N_CLS), dtype=np.float32)
    for c in range(N_CORES):
        out[meta["core_idx"][c]] = res.results[c]["out"].T
    return out


# revision 6
# speedup vs baseline: 1.2037x; 1.2037x over previous
import sys

sys.path.insert(0, "/opt/trn_rl_repo")

import numpy as np
import ml_dtypes

# ---- problem constants (hardcoded from the nn_LocalAggregator spec) ----
PC_MIN = np.array([-40.0, -40.0, -1.0], dtype=np.float32)
GRID = np.float32(0.4)
SCALE_MULT = np.float32(3.0)
N_PTS, N_GAUSS, N_CLS = 16384, 4096, 18
N_CORES = 8
NPC = N_PTS // N_CORES          # 2048 points per core
BLK = 512                       # point block (matmul free dim)
NBLK = NPC // BLK               # 4
P = 128                         # partitions / gaussians per tile
BIG = np.float64(1024.0)        # mask penalty (one violated axis is enough)
DUMMY_BIAS = -30000.0           # exp(-30000) == 0 exactly in fp32
KQ = 9                          # quadratic+linear monomial features
NSPLIT = [(0, 0), (0, 1), (1, 0), (1, 1), (0, 2), (2, 0)]  # bf16 split combos
KQR = KQ * len(NSPLIT)          # quad rows after splitting
KZ = 16                         # z voxel range
Y_CULL = True

BF16 = ml_dtypes.bfloat16

# module global for test harness introspection (exec time etc.)
LAST_RESULTS = None


def _split3(x):
    """float64 array -> 3 bf16 levels whose sum ~= x to ~24 bits."""
    a = x.astype(BF16)
    r = x - a.astype(np.float64)
    b = r.astype(BF16)
    r = r - b.astype(np.float64)
    c = r.astype(BF16)
    return a, b, c


def _prep(pts, means3D, opacities, semantics, scales, cov3D):
    """Host-side O(N+M) prep: sharding, features, coefficient tables."""
    p = np.asarray(pts[0], dtype=np.float32)          # [N,3]
    mu = np.asarray(means3D[0], dtype=np.float32)     # [M,3]
    opa = np.asarray(opacities[0], dtype=np.float32)  # [M]
    sem = np.asarray(semantics[0], dtype=np.float32)  # [M,C]
    sc = np.asarray(scales[0], dtype=np.float32)      # [M,3]
    cov = np.asarray(cov3D[0], dtype=np.float32)      # [M,3,3]

    # integer voxel coords / radii -- fp32 ops exactly as the reference
    p_int = ((p - PC_MIN) / GRID).astype(np.int32)
    m_int = ((mu - PC_MIN) / GRID).astype(np.int32)
    radii = np.ceil(sc.max(axis=-1) * SCALE_MULT / GRID).astype(np.int32)

    # symmetric precision entries, same picks as reference
    cxx = cov[:, 0, 0].astype(np.float64)
    cyy = cov[:, 1, 1].astype(np.float64)
    czz = cov[:, 2, 2].astype(np.float64)
    cxy = cov[:, 0, 1].astype(np.float64)
    cyz = cov[:, 1, 2].astype(np.float64)
    cxz = cov[:, 0, 2].astype(np.float64)
    with np.errstate(divide="ignore"):
        lnopa = np.log(opa.astype(np.float64))
    lnopa = np.maximum(lnopa, -20000.0)

    # ---- shard points: equal x-chunks, y-sorted inside each core ----
    order_x = np.argsort(p[:, 0], kind="stable")
    core_idx = []
    for c in range(N_CORES):
        idx = order_x[c * NPC:(c + 1) * NPC]
        idx = idx[np.argsort(p[idx, 1], kind="stable")]
        core_idx.append(idx)

    # ---- per-core gaussian subsets (x-reach cull), y-sorted ----
    core_gsel = []
    for c in range(N_CORES):
        vx = p_int[core_idx[c], 0]
        m = (m_int[:, 0] >= vx.min() - radii) & (m_int[:, 0] <= vx.max() + radii)
        gs = np.nonzero(m)[0]
        gs = gs[np.argsort(m_int[gs, 1], kind="stable")]
        core_gsel.append(gs)
    g_pad = P * int(np.ceil(max(len(g) for g in core_gsel) / P))
    n_gt = g_pad // P

    # ---- structural sizes shared across cores (SPMD) ----
    kx = 0
    ky = 0
    for c in range(N_CORES):
        vx = p_int[core_idx[c], 0]
        kx = max(kx, int(vx.max() - vx.min()) + 1)
        vy = p_int[core_idx[c], 1]
        for b in range(NBLK):
            vyb = vy[b * BLK:(b + 1) * BLK]
            ky = max(ky, int(vyb.max() - vyb.min()) + 1)
    ktot = KQR + kx + ky + KZ

    # ---- pair lists (g-tile x block), y-culled, padded across cores ----
    core_pairs = []   # per core: list of (b, t) with t = -1 for dummy
    counts = np.zeros((N_CORES, NBLK), dtype=np.int64)
    per_core_tb = []
    for c in range(N_CORES):
        gs = core_gsel[c]
        vy = p_int[core_idx[c], 1]
        tb = []
        for b in range(NBLK):
            vyb = vy[b * BLK:(b + 1) * BLK]
            ylo, yhi = int(vyb.min()), int(vyb.max())
            lst = []
            for t in range(n_gt):
                gg = gs[t * P:(t + 1) * P]
                if len(gg) == 0:
                    continue
                gl = (m_int[gg, 1] - radii[gg]).min()
                gh = (m_int[gg, 1] + radii[gg]).max()
                if (not Y_CULL) or (gl <= yhi and gh >= ylo):
                    lst.append(t)
            tb.append(lst)
            counts[c, b] = len(lst)
        per_core_tb.append(tb)
    npb = counts.max(axis=0)          # padded per-block pair counts
    npair = int(npb.sum())
    pair_block = []                    # baked structure, same for all cores
    for b in range(NBLK):
        pair_block += [b] * int(npb[b])

    # ---- per-core device arrays ----
    in_maps = []
    for c in range(N_CORES):
        idx = core_idx[c]
        gs = core_gsel[c]
        vx = p_int[idx, 0]
        vy = p_int[idx, 1]
        vz = p_int[idx, 2]
        vx_lo = int(vx.min())
        pc = p[idx].astype(np.float64)     # [NPC,3]

        # per-block centers
        centers = np.stack([pc[b * BLK:(b + 1) * BLK].mean(axis=0)
                            for b in range(NBLK)])   # [NBLK,3]
        ylos = [int(vy[b * BLK:(b + 1) * BLK].min()) for b in range(NBLK)]

        # ---- feature matrix FEAT [ktot, NPC] bf16 ----
        feat = np.zeros((ktot, NPC), dtype=BF16)
        for b in range(NBLK):
            cols = slice(b * BLK, (b + 1) * BLK)
            d = pc[cols] - centers[b]                  # [BLK,3] float64
            x, y, z = d[:, 0], d[:, 1], d[:, 2]
            q = np.stack([x * x, y * y, z * z, x * y, y * z, x * z, x, y, z])
            qs = _split3(q)                            # 3 x [KQ, BLK]
            for f in range(KQ):
                for k, (i, _) in enumerate(NSPLIT):
                    feat[f * len(NSPLIT) + k, cols] = qs[i][f]
            # one-hots
            rx = KQR + (vx[cols] - vx_lo)
            ryy = KQR + kx + (vy[cols] - ylos[b])
            rz = KQR + kx + ky + vz[cols]
            ar = np.arange(b * BLK, (b + 1) * BLK)
            feat[rx, ar] = BF16(1)
            feat[ryy, ar] = BF16(1)
            feat[rz, ar] = BF16(1)

        # ---- per-pair stationaries / biases / sem tiles ----
        stat = np.zeros((npair, ktot, P), dtype=BF16)
        bias = np.full((P, npair), DUMMY_BIAS, dtype=np.float32)
        semt = np.zeros((P, npair * N_CLS), dtype=np.float32)
        pi = 0
        for b in range(NBLK):
            lst = per_core_tb[c][b]
            for j in range(int(npb[b])):
                if j < len(lst):
                    t = lst[j]
                    gg = gs[t * P:(t + 1) * P]
                    ng = len(gg)
                    mup = mu[gg].astype(np.float64) - centers[b]  # [ng,3]
                    mx, my, mz = mup[:, 0], mup[:, 1], mup[:, 2]
                    gxx, gyy, gzz = cxx[gg], cyy[gg], czz[gg]
                    gxy, gyz, gxz = cxy[gg], cyz[gg], cxz[gg]
                    hx = gxx * mx + gxy * my + gxz * mz
                    hy = gxy * mx + gyy * my + gyz * mz
                    hz = gxz * mx + gyz * my + gzz * mz
                    gq = np.stack([-0.5 * gxx, -0.5 * gyy, -0.5 * gzz,
                                   -gxy, -gyz, -gxz, hx, hy, hz])  # [KQ,ng]
                    gsp = _split3(gq)
                    for f in range(KQ):
                        for k, (_, jj) in enumerate(NSPLIT):
                            stat[pi, f * len(NSPLIT) + k, :ng] = gsp[jj][f]
                    # interval tables (0 within reach, -BIG outside)
                    vv = np.arange(kx)[:, None] + vx_lo
                    out_x = np.abs(vv - m_int[gg, 0][None, :]) > radii[gg][None, :]
                    stat[pi, KQR:KQR + kx, :ng] = np.where(out_x, -BIG, 0.0).astype(BF16)
                    vv = np.arange(ky)[:, None] + ylos[b]
                    out_y = np.abs(vv - m_int[gg, 1][None, :]) > radii[gg][None, :]
                    stat[pi, KQR + kx:KQR + kx + ky, :ng] = np.where(out_y, -BIG, 0.0).astype(BF16)
                    vv = np.arange(KZ)[:, None]
                    out_z = np.abs(vv - m_int[gg, 2][None, :]) > radii[gg][None, :]
                    stat[pi, KQR + kx + ky:, :ng] = np.where(out_z, -BIG, 0.0).astype(BF16)
                    # bias: -0.5 mu'^T C mu' + ln(opa)
                    quad = (gxx * mx * mx + gyy * my * my + gzz * mz * mz
                            + 2 * gxy * mx * my + 2 * gyz * my * mz + 2 * gxz * mx * mz)
                    bias[:ng, pi] = (-0.5 * quad + lnopa[gg]).astype(np.float32)
                    semt[:ng, pi * N_CLS:(pi + 1) * N_CLS] = sem[gg]
                pi += 1

        # chunk layout: stat rows as [kchunks, 128, P] padded
        nchunks = int(np.ceil(ktot / P))
        kpad = nchunks * P
        featp = np.zeros((kpad, NPC), dtype=BF16)
        featp[:ktot] = feat
        statp = np.zeros((npair, kpad, P), dtype=BF16)
        statp[:, :ktot] = stat
        statt = statp.reshape(npair, nchunks, P, P)  # [pair, chunk, krow, g]
        statt = statt.transpose(2, 1, 0, 3).reshape(P, nchunks * npair * P)
        # rows = krow partition (128), cols = (chunk, pair, gauss)

        in_maps.append({
            "feat": featp.reshape(nchunks, P, NPC).transpose(1, 0, 2).reshape(P, nchunks * NPC),
            "stat": statt,
            "bias": bias,
            "semt": semt,
        })

    meta = dict(npair=npair, pair_block=pair_block, nchunks=nchunks,
                core_idx=core_idx, npb=npb)
    return in_maps, meta


def _build_nc(npair, pair_block, nchunks):
    import concourse.bass as bass  # noqa: F401
    import concourse.mybir as mybir
    import concourse.tile as tile
    from concourse import bacc

    f32 = mybir.dt.float32
    bf16 = mybir.dt.bfloat16

    nc = bacc.Bacc("TRN2", target_bir_lowering=False, debug=False,
                   num_devices=N_CORES)
    feat_d = nc.dram_tensor("feat", [P, nchunks * NPC], bf16, kind="ExternalInput")
    stat_d = nc.dram_tensor("stat", [P, nchunks * npair * P], bf16, kind="ExternalInput")
    bias_d = nc.dram_tensor("bias", [P, npair], f32, kind="ExternalInput")
    semt_d = nc.dram_tensor("semt", [P, npair * N_CLS], f32, kind="ExternalInput")
    out_d = nc.dram_tensor("out", [N_CLS, NPC], f32, kind="ExternalOutput")

    # first/last pair index per block for psum accumulate flags
    first = {}
    last = {}
    for i, b in enumerate(pair_block):
        first.setdefault(b, i)
        last[b] = i

    with tile.TileContext(nc) as tc:
        with (
            tc.tile_pool(name="resident", bufs=1) as res_pool,
            tc.tile_pool(name="wpool", bufs=3) as w_pool,
            tc.tile_pool(name="pw", bufs=3, space="PSUM") as pw_pool,
            tc.tile_pool(name="lgp", bufs=1, space="PSUM") as lg_pool,
        ):
            feat_s = res_pool.tile([P, nchunks * NPC], bf16, name="feat_s")
            stat_s = res_pool.tile([P, nchunks * npair * P], bf16, name="stat_s")
            bias_s = res_pool.tile([P, npair], f32, name="bias_s")
            semt_s = res_pool.tile([P, npair * N_CLS], f32, name="semt_s")
            out_s = res_pool.tile([N_CLS, NPC], f32, name="out_s")

            # stage inputs: few chunky DMAs, round-robined across queues so
            # the issuing engines don't serialize (Sync was 57% busy before)
            nc.gpsimd.dma_start(out=bias_s[:], in_=bias_d[:])
            nc.gpsimd.dma_start(out=semt_s[:], in_=semt_d[:])
            for ch in range(nchunks):
                nc.sync.dma_start(
                    out=feat_s[:, ch * NPC:(ch + 1) * NPC],
                    in_=feat_d[:, ch * NPC:(ch + 1) * NPC])
            nstat = nchunks * npair
            engines = [nc.sync, nc.gpsimd, nc.scalar]
            ngrp = min(6, nstat)
            bounds = [nstat * g // ngrp for g in range(ngrp + 1)]
            for g in range(ngrp):
                lo, hi = bounds[g] * P, bounds[g + 1] * P
                engines[g % len(engines)].dma_start(
                    out=stat_s[:, lo:hi], in_=stat_d[:, lo:hi])

            lg = [lg_pool.tile([N_CLS, BLK], f32, name=f"lg{b}")
                  for b in range(NBLK)]

            for i, b in enumerate(pair_block):
                cols = slice(b * BLK, (b + 1) * BLK)
                pw = pw_pool.tile([P, BLK], f32, name="pw")
                for ch in range(nchunks):
                    lhs = stat_s[:, (ch * npair + i) * P:(ch * npair + i + 1) * P]
                    rhs = feat_s[:, ch * NPC + b * BLK: ch * NPC + (b + 1) * BLK]
                    nc.tensor.matmul(out=pw[:], lhsT=lhs, rhs=rhs,
                                     start=(ch == 0), stop=(ch == nchunks - 1))
                w = w_pool.tile([P, BLK], f32, name="w")
                nc.scalar.activation(w[:], pw[:],
                                     mybir.ActivationFunctionType.Exp,
                                     bias=bias_s[:, i:i + 1])
                nc.tensor.matmul(out=lg[b][:],
                                 lhsT=semt_s[:, i * N_CLS:(i + 1) * N_CLS],
                                 rhs=w[:],
                                 start=(first[b] == i), stop=(last[b] == i))

            for b in range(NBLK):
                nc.vector.tensor_copy(out_s[:, b * BLK:(b + 1) * BLK], lg[b][:])
            nc.sync.dma_start(out=out_d[:], in_=out_s[:])

    nc.compile()
    return nc


def kernel(pts, means3D, opacities, semantics, scales, cov3D):
    global LAST_RESULTS
    from concourse.bass_utils import run_bass_kernel_spmd

    in_maps, meta = _prep(pts, means3D, opacities, semantics, scales, cov3D)
    nc = _build_nc(meta["npair"], meta["pair_block"], meta["nchunks"])
    res = run_bass_kernel_spmd(nc, in_maps, core_ids=list(range(N_CORES)))
    LAST_RESULTS = res

    out = np.empty((N_PTS, N_CLS), dtype=np.float32)
    for c in range(N_CORES):
        out[meta["core_idx"][c]] = res.results[c]["out"].T
    return out


# revision 13
# speedup vs baseline: 1.5412x; 1.2804x over previous
import sys

sys.path.insert(0, "/opt/trn_rl_repo")

import numpy as np
import ml_dtypes

# ---- problem constants (hardcoded from the nn_LocalAggregator spec) ----
PC_MIN = np.array([-40.0, -40.0, -1.0], dtype=np.float32)
GRID = np.float32(0.4)
SCALE_MULT = np.float32(3.0)
N_PTS, N_GAUSS, N_CLS = 16384, 4096, 18
N_CORES = 8
NPC = N_PTS // N_CORES          # 2048 points per core
BLK = 512                       # point block (matmul free dim)
NBLK = NPC // BLK               # 4
P = 128                         # partitions / gaussians per tile
BIG = np.float64(1024.0)        # mask penalty (one violated axis is enough)
DUMMY_BIAS = -30000.0           # exp(-30000) == 0 exactly in fp32
KQ = 9                          # quadratic+linear monomial features
NSPLIT = [(0, 0), (0, 1), (1, 0), (1, 1), (0, 2), (2, 0)]  # bf16 split combos
KQR = KQ * len(NSPLIT)          # quad rows after splitting
KZ = 16                         # z voxel range
Y_CULL = True

BF16 = ml_dtypes.bfloat16

# module global for test harness introspection (exec time etc.)
LAST_RESULTS = None


def _split3(x):
    """float64 array -> 3 bf16 levels whose sum ~= x to ~24 bits."""
    a = x.astype(BF16)
    r = x - a.astype(np.float64)
    b = r.astype(BF16)
    r = r - b.astype(np.float64)
    c = r.astype(BF16)
    return a, b, c


def _prep(pts, means3D, opacities, semantics, scales, cov3D):
    """Host-side O(N+M) prep: sharding, features, coefficient tables."""
    p = np.asarray(pts[0], dtype=np.float32)          # [N,3]
    mu = np.asarray(means3D[0], dtype=np.float32)     # [M,3]
    opa = np.asarray(opacities[0], dtype=np.float32)  # [M]
    sem = np.asarray(semantics[0], dtype=np.float32)  # [M,C]
    sc = np.asarray(scales[0], dtype=np.float32)      # [M,3]
    cov = np.asarray(cov3D[0], dtype=np.float32)      # [M,3,3]

    # integer voxel coords / radii -- fp32 ops exactly as the reference
    p_int = ((p - PC_MIN) / GRID).astype(np.int32)
    m_int = ((mu - PC_MIN) / GRID).astype(np.int32)
    radii = np.ceil(sc.max(axis=-1) * SCALE_MULT / GRID).astype(np.int32)

    # symmetric precision entries, same picks as reference
    cxx = cov[:, 0, 0].astype(np.float64)
    cyy = cov[:, 1, 1].astype(np.float64)
    czz = cov[:, 2, 2].astype(np.float64)
    cxy = cov[:, 0, 1].astype(np.float64)
    cyz = cov[:, 1, 2].astype(np.float64)
    cxz = cov[:, 0, 2].astype(np.float64)
    with np.errstate(divide="ignore"):
        lnopa = np.log(opa.astype(np.float64))
    lnopa = np.maximum(lnopa, -20000.0)

    # ---- shard points: equal x-chunks, y-sorted inside each core ----
    order_x = np.argsort(p[:, 0], kind="stable")
    core_idx = []
    for c in range(N_CORES):
        idx = order_x[c * NPC:(c + 1) * NPC]
        idx = idx[np.argsort(p[idx, 1], kind="stable")]
        core_idx.append(idx)

    # ---- per-core gaussian subsets (x-reach cull), y-sorted ----
    core_gsel = []
    for c in range(N_CORES):
        vx = p_int[core_idx[c], 0]
        m = (m_int[:, 0] >= vx.min() - radii) & (m_int[:, 0] <= vx.max() + radii)
        gs = np.nonzero(m)[0]
        gs = gs[np.argsort(m_int[gs, 1], kind="stable")]
        core_gsel.append(gs)
    g_pad = P * int(np.ceil(max(len(g) for g in core_gsel) / P))
    n_gt = g_pad // P

    # ---- structural sizes shared across cores (SPMD) ----
    kx = 0
    ky = 0
    for c in range(N_CORES):
        vx = p_int[core_idx[c], 0]
        kx = max(kx, int(vx.max() - vx.min()) + 1)
        vy = p_int[core_idx[c], 1]
        for b in range(NBLK):
            vyb = vy[b * BLK:(b + 1) * BLK]
            ky = max(ky, int(vyb.max() - vyb.min()) + 1)
    ktot = KQR + kx + ky + KZ

    # ---- pair lists (g-tile x block), y-culled, padded across cores ----
    core_pairs = []   # per core: list of (b, t) with t = -1 for dummy
    counts = np.zeros((N_CORES, NBLK), dtype=np.int64)
    per_core_tb = []
    for c in range(N_CORES):
        gs = core_gsel[c]
        vy = p_int[core_idx[c], 1]
        tb = []
        for b in range(NBLK):
            vyb = vy[b * BLK:(b + 1) * BLK]
            ylo, yhi = int(vyb.min()), int(vyb.max())
            lst = []
            for t in range(n_gt):
                gg = gs[t * P:(t + 1) * P]
                if len(gg) == 0:
                    continue
                gl = (m_int[gg, 1] - radii[gg]).min()
                gh = (m_int[gg, 1] + radii[gg]).max()
                if (not Y_CULL) or (gl <= yhi and gh >= ylo):
                    lst.append(t)
            tb.append(lst)
            counts[c, b] = len(lst)
        per_core_tb.append(tb)
    npb = counts.max(axis=0)          # padded per-block pair counts
    npair = int(npb.sum())
    pair_block = []                    # baked structure, same for all cores
    for b in range(NBLK):
        pair_block += [b] * int(npb[b])

    # ---- per-core device arrays ----
    in_maps = []
    for c in range(N_CORES):
        idx = core_idx[c]
        gs = core_gsel[c]
        vx = p_int[idx, 0]
        vy = p_int[idx, 1]
        vz = p_int[idx, 2]
        vx_lo = int(vx.min())
        pc = p[idx].astype(np.float64)     # [NPC,3]

        # per-block centers
        centers = np.stack([pc[b * BLK:(b + 1) * BLK].mean(axis=0)
                            for b in range(NBLK)])   # [NBLK,3]
        ylos = [int(vy[b * BLK:(b + 1) * BLK].min()) for b in range(NBLK)]

        # ---- feature matrix FEAT [ktot, NPC] bf16 ----
        feat = np.zeros((ktot, NPC), dtype=BF16)
        for b in range(NBLK):
            cols = slice(b * BLK, (b + 1) * BLK)
            d = pc[cols] - centers[b]                  # [BLK,3] float64
            x, y, z = d[:, 0], d[:, 1], d[:, 2]
            q = np.stack([x * x, y * y, z * z, x * y, y * z, x * z, x, y, z])
            qs = _split3(q)                            # 3 x [KQ, BLK]
            for f in range(KQ):
                for k, (i, _) in enumerate(NSPLIT):
                    feat[f * len(NSPLIT) + k, cols] = qs[i][f]
            # one-hots
            rx = KQR + (vx[cols] - vx_lo)
            ryy = KQR + kx + (vy[cols] - ylos[b])
            rz = KQR + kx + ky + vz[cols]
            ar = np.arange(b * BLK, (b + 1) * BLK)
            feat[rx, ar] = BF16(1)
            feat[ryy, ar] = BF16(1)
            feat[rz, ar] = BF16(1)

        # ---- per-pair stationaries / biases / sem tiles ----
        stat = np.zeros((npair, ktot, P), dtype=BF16)
        bias = np.full((P, npair), DUMMY_BIAS, dtype=np.float32)
        semt = np.zeros((P, npair * N_CLS), dtype=np.float32)
        pi = 0
        for b in range(NBLK):
            lst = per_core_tb[c][b]
            for j in range(int(npb[b])):
                if j < len(lst):
                    t = lst[j]
                    gg = gs[t * P:(t + 1) * P]
                    ng = len(gg)
                    mup = mu[gg].astype(np.float64) - centers[b]  # [ng,3]
                    mx, my, mz = mup[:, 0], mup[:, 1], mup[:, 2]
                    gxx, gyy, gzz = cxx[gg], cyy[gg], czz[gg]
                    gxy, gyz, gxz = cxy[gg], cyz[gg], cxz[gg]
                    hx = gxx * mx + gxy * my + gxz * mz
                    hy = gxy * mx + gyy * my + gyz * mz
                    hz = gxz * mx + gyz * my + gzz * mz
                    gq = np.stack([-0.5 * gxx, -0.5 * gyy, -0.5 * gzz,
                                   -gxy, -gyz, -gxz, hx, hy, hz])  # [KQ,ng]
                    gsp = _split3(gq)
                    for f in range(KQ):
                        for k, (_, jj) in enumerate(NSPLIT):
                            stat[pi, f * len(NSPLIT) + k, :ng] = gsp[jj][f]
                    # interval tables (0 within reach, -BIG outside)
                    vv = np.arange(kx)[:, None] + vx_lo
                    out_x = np.abs(vv - m_int[gg, 0][None, :]) > radii[gg][None, :]
                    stat[pi, KQR:KQR + kx, :ng] = np.where(out_x, -BIG, 0.0).astype(BF16)
                    vv = np.arange(ky)[:, None] + ylos[b]
                    out_y = np.abs(vv - m_int[gg, 1][None, :]) > radii[gg][None, :]
                    stat[pi, KQR + kx:KQR + kx + ky, :ng] = np.where(out_y, -BIG, 0.0).astype(BF16)
                    vv = np.arange(KZ)[:, None]
                    out_z = np.abs(vv - m_int[gg, 2][None, :]) > radii[gg][None, :]
                    stat[pi, KQR + kx + ky:, :ng] = np.where(out_z, -BIG, 0.0).astype(BF16)
                    # bias: -0.5 mu'^T C mu' + ln(opa)
                    quad = (gxx * mx * mx + gyy * my * my + gzz * mz * mz
                            + 2 * gxy * mx * my + 2 * gyz * my * mz + 2 * gxz * mx * mz)
                    bias[:ng, pi] = (-0.5 * quad + lnopa[gg]).astype(np.float32)
                    semt[:ng, pi * N_CLS:(pi + 1) * N_CLS] = sem[gg]
                pi += 1

        # chunk layout: stat rows as [kchunks, 128, P] padded
        nchunks = int(np.ceil(ktot / P))
        kpad = nchunks * P
        featp = np.zeros((kpad, NPC), dtype=BF16)
        featp[:ktot] = feat
        statp = np.zeros((npair, kpad, P), dtype=BF16)
        statp[:, :ktot] = stat
        statt = statp.reshape(npair, nchunks, P, P)  # [pair, chunk, krow, g]
        statt = statt.transpose(2, 1, 0, 3).reshape(P, nchunks * npair * P)
        # rows = krow partition (128), cols = (chunk, pair, gauss)

        in_maps.append({
            "feat": featp.reshape(nchunks, P, NPC).transpose(1, 0, 2).reshape(P, nchunks * NPC),
            "stat": statt,
            "bias": bias,
            "semt": semt,
        })

    meta = dict(npair=npair, pair_block=pair_block, nchunks=nchunks,
                core_idx=core_idx, npb=npb)
    return in_maps, meta


def _build_nc(npair, pair_block, nchunks):
    import concourse.bass as bass  # noqa: F401
    import concourse.mybir as mybir
    import concourse.tile as tile
    from concourse import bacc

    f32 = mybir.dt.float32
    bf16 = mybir.dt.bfloat16

    nc = bacc.Bacc("TRN2", target_bir_lowering=False, debug=False,
                   num_devices=N_CORES)
    feat_d = nc.dram_tensor("feat", [P, nchunks * NPC], bf16, kind="ExternalInput")
    stat_d = nc.dram_tensor("stat", [P, nchunks * npair * P], bf16, kind="ExternalInput")
    bias_d = nc.dram_tensor("bias", [P, npair], f32, kind="ExternalInput")
    f32r = mybir.dt.float32r
    semt_d = nc.dram_tensor("semt", [P, npair * N_CLS], f32r, kind="ExternalInput")
    out_d = nc.dram_tensor("out", [N_CLS, NPC], f32, kind="ExternalOutput")

    # first/last pair index per block for psum accumulate flags
    first = {}
    last = {}
    for i, b in enumerate(pair_block):
        first.setdefault(b, i)
        last[b] = i

    with tile.TileContext(nc) as tc:
        with (
            tc.tile_pool(name="resident", bufs=1) as res_pool,
            tc.tile_pool(name="wpool", bufs=3) as w_pool,
            tc.tile_pool(name="pw", bufs=3, space="PSUM") as pw_pool,
            tc.tile_pool(name="lgp", bufs=1, space="PSUM") as lg_pool,
        ):
            feat_s = res_pool.tile([P, nchunks * NPC], bf16, name="feat_s")
            stat_s = res_pool.tile([P, nchunks * npair * P], bf16, name="stat_s")
            bias_s = res_pool.tile([P, npair], f32, name="bias_s")
            semt_s = res_pool.tile([P, npair * N_CLS], f32r, name="semt_s")
            out_s = res_pool.tile([N_CLS, NPC], f32, name="out_s")

            # stage inputs: few chunky DMAs, round-robined across queues so
            # the issuing engines don't serialize (Sync was 57% busy before)
            nc.scalar.dma_start(out=bias_s[:], in_=bias_d[:])
            nc.scalar.dma_start(out=semt_s[:], in_=semt_d[:])
            for ch in range(nchunks):
                nc.sync.dma_start(
                    out=feat_s[:, ch * NPC:(ch + 1) * NPC],
                    in_=feat_d[:, ch * NPC:(ch + 1) * NPC])
            nstat = nchunks * npair
            engines = [nc.sync, nc.scalar]
            ngrp = min(6, nstat)
            bounds = [nstat * g // ngrp for g in range(ngrp + 1)]
            for g in range(ngrp):
                lo, hi = bounds[g] * P, bounds[g + 1] * P
                engines[g % len(engines)].dma_start(
                    out=stat_s[:, lo:hi], in_=stat_d[:, lo:hi])

            lg = [lg_pool.tile([N_CLS, BLK], f32, name=f"lg{b}")
                  for b in range(NBLK)]

            for i, b in enumerate(pair_block):
                cols = slice(b * BLK, (b + 1) * BLK)
                pw = pw_pool.tile([P, BLK], f32, name="pw")
                for ch in range(nchunks):
                    lhs = stat_s[:, (ch * npair + i) * P:(ch * npair + i + 1) * P]
                    rhs = feat_s[:, ch * NPC + b * BLK: ch * NPC + (b + 1) * BLK]
                    nc.tensor.matmul(out=pw[:], lhsT=lhs, rhs=rhs,
                                     start=(ch == 0), stop=(ch == nchunks - 1))
                w = w_pool.tile([P, BLK], f32r, name="w")
                nc.scalar.activation(w[:], pw[:],
                                     mybir.ActivationFunctionType.Exp,
                                     bias=bias_s[:, i:i + 1])
                nc.tensor.matmul(out=lg[b][:],
                                 lhsT=semt_s[:, i * N_CLS:(i + 1) * N_CLS],
                                 rhs=w[:],
                                 start=(first[b] == i), stop=(last[b] == i))

            for b in range(NBLK):
                nc.vector.tensor_copy(out_s[:, b * BLK:(b + 1) * BLK], lg[b][:])
            nc.sync.dma_start(out=out_d[:], in_=out_s[:])

    nc.compile()
    return nc


def kernel(pts, means3D, opacities, semantics, scales, cov3D):
    global LAST_RESULTS
    from concourse.bass_utils import run_bass_kernel_spmd

    in_maps, meta = _prep(pts, means3D, opacities, semantics, scales, cov3D)
    nc = _build_nc(meta["npair"], meta["pair_block"], meta["nchunks"])
    res = run_bass_kernel_spmd(nc, in_maps, core_ids=list(range(N_CORES)))
    LAST_RESULTS = res

    out = np.empty((N_PTS, N_CLS), dtype=np.float32)
    for c in range(N_CORES):
        out[meta["core_idx"][c]] = res.results[c]["out"].T
    return out
